# revision 1
# baseline (speedup 1.0000x reference)
"""InfoNCE loss kernel for Trainium2 (8 NeuronCores, Bass/Tile).

Strategy (data-parallel over batch, per sharding hint):
  - batch 16384 split 8 ways -> 2048 items per core, processed as 16 tiles
    of 128 items (one item per SBUF partition).
  - per tile: one indirect DMA gathers the 22 embedding rows each item
    needs (target, context, 20 negatives) -> SBUF [128, 22*128] f32.
  - DVE computes products (broadcast target over the 21 "other" rows) and
    reduces over D=128 -> scores [128, 21].
  - ACT computes exp((s - max)/T) with free-dim accumulate, then ln.
  - per-item loss = ln(sum exp) + (max - s_pos)/T, accumulated per
    partition; each core outputs its [128,1] partial sums.
  - host sums the 8x128 partials / 16384.
"""

import os
import sys

for _p in ("/opt/trn_rl_repo", "/root/.axon_site/_ro/trn_rl_repo"):
    if os.path.isdir(_p):
        sys.path.insert(0, _p)

import numpy as np

import concourse.tile as tile
from concourse import bacc, bass, mybir
from concourse.bass import IndirectOffsetOnAxis
from concourse.bass_utils import run_bass_kernel_spmd

NUM_NODES = 100000
DIM = 128
BATCH = 16384
NUM_NEG = 20
TEMPERATURE = 0.07

N_CORES = 8
P = 128
ITEMS_PER_CORE = BATCH // N_CORES  # 2048
TILES = ITEMS_PER_CORE // P  # 16
J = 2 + NUM_NEG  # 22 gathered rows per item
NJ = 1 + NUM_NEG  # 21 score columns (ctx + 20 negs)
INV_T = 1.0 / TEMPERATURE

f32 = mybir.dt.float32
i32 = mybir.dt.int32

_cached_nc = None
_last_results = None


def _build():
    global _cached_nc
    if _cached_nc is not None:
        return _cached_nc

    nc = bacc.Bacc(None, target_bir_lowering=False)
    emb = nc.declare_dram_parameter("emb", [NUM_NODES, DIM], f32, isOutput=False)
    idx = nc.declare_dram_parameter("idx", [P, TILES * J], i32, isOutput=False)
    out = nc.declare_dram_parameter("out", [P, 1], f32, isOutput=True)

    with tile.TileContext(nc) as tc:
        with (
            tc.tile_pool(name="main", bufs=1) as sp,
            tc.tile_pool(name="g", bufs=2) as gp,
            tc.tile_pool(name="w", bufs=2) as wp,
        ):
            idx_t = sp.tile([P, TILES * J], i32)
            nc.sync.dma_start(out=idx_t[:], in_=idx[:])
            contribs = sp.tile([P, TILES], f32)

            for t in range(TILES):
                G = gp.tile([P, J * DIM], f32, tag="G")
                # HW only honors one offset per partition per indirect DMA
                # (scatter_add-style [P,1] offset APs) — one call per role j.
                for j in range(J):
                    nc.gpsimd.indirect_dma_start(
                        out=G[:, j * DIM : (j + 1) * DIM],
                        out_offset=None,
                        in_=emb[:],
                        in_offset=IndirectOffsetOnAxis(
                            ap=idx_t[:, t * J + j : t * J + j + 1], axis=0
                        ),
                    )
                # scores[p, j] = dot(G[p, 0, :], G[p, j+1, :]) for j in 0..20
                prod = wp.tile([P, NJ * DIM], f32, tag="prod")
                rest3 = G[:, DIM:].rearrange("p (j d) -> p j d", j=NJ)
                tgt_b = G[:, 0:DIM].unsqueeze(1).to_broadcast([P, NJ, DIM])
                nc.vector.tensor_tensor(
                    out=prod[:].rearrange("p (j d) -> p j d", j=NJ),
                    in0=rest3,
                    in1=tgt_b,
                    op=mybir.AluOpType.mult,
                )
                scores = wp.tile([P, NJ], f32, tag="scores")
                nc.vector.tensor_reduce(
                    out=scores[:],
                    in_=prod[:].rearrange("p (j d) -> p j d", j=NJ),
                    axis=mybir.AxisListType.X,
                    op=mybir.AluOpType.add,
                )
                mx = wp.tile([P, 1], f32, tag="mx")
                nc.vector.tensor_reduce(
                    out=mx[:],
                    in_=scores[:],
                    axis=mybir.AxisListType.X,
                    op=mybir.AluOpType.max,
                )
                negm = wp.tile([P, 1], f32, tag="negm")
                nc.vector.tensor_scalar_mul(out=negm[:], in0=mx[:], scalar1=-INV_T)
                etile = wp.tile([P, NJ], f32, tag="etile")
                ssum = wp.tile([P, 1], f32, tag="ssum")
                nc.scalar.activation(
                    out=etile[:],
                    in_=scores[:],
                    func=mybir.ActivationFunctionType.Exp,
                    bias=negm[:, 0:1],
                    scale=INV_T,
                    accum_out=ssum[:],
                )
                lns = wp.tile([P, 1], f32, tag="lns")
                nc.scalar.activation(
                    out=lns[:],
                    in_=ssum[:],
                    func=mybir.ActivationFunctionType.Ln,
                )
                # contrib = ln(sum) + (mx - s_pos) * (1/T)
                d1 = wp.tile([P, 1], f32, tag="d1")
                nc.vector.tensor_tensor(
                    out=d1[:],
                    in0=mx[:],
                    in1=scores[:, 0:1],
                    op=mybir.AluOpType.subtract,
                )
                nc.vector.scalar_tensor_tensor(
                    out=contribs[:, t : t + 1],
                    in0=d1[:],
                    scalar=INV_T,
                    in1=lns[:],
                    op0=mybir.AluOpType.mult,
                    op1=mybir.AluOpType.add,
                )

            result = sp.tile([P, 1], f32)
            nc.vector.tensor_reduce(
                out=result[:],
                in_=contribs[:],
                axis=mybir.AxisListType.X,
                op=mybir.AluOpType.add,
            )
            nc.sync.dma_start(out=out[:], in_=result[:])

    nc.compile()
    _cached_nc = nc
    return nc


def kernel(embeddings, targets, contexts, negatives):
    global _last_results
    nc = _build()

    emb = np.ascontiguousarray(np.asarray(embeddings, dtype=np.float32))
    t32 = np.asarray(targets).astype(np.int32).reshape(BATCH, 1)
    c32 = np.asarray(contexts).astype(np.int32).reshape(BATCH, 1)
    n32 = np.asarray(negatives).astype(np.int32).reshape(BATCH, NUM_NEG)
    idx_all = np.concatenate([t32, c32, n32], axis=1)  # [BATCH, 22]

    in_maps = []
    for c in range(N_CORES):
        sl = idx_all[c * ITEMS_PER_CORE : (c + 1) * ITEMS_PER_CORE]  # [2048, 22]
        # partition p holds items {t*128+p}: SBUF layout [128, 16*22]
        arr = np.ascontiguousarray(
            sl.reshape(TILES, P, J).transpose(1, 0, 2).reshape(P, TILES * J)
        )
        in_maps.append({"emb": emb, "idx": arr})

    trace = bool(int(os.environ.get("KERNEL_TRACE", "0")))
    res = run_bass_kernel_spmd(
        nc, in_maps, list(range(N_CORES)), trace=trace
    )
    _last_results = res

    total = 0.0
    for c in range(N_CORES):
        total += float(res.results[c]["out"].reshape(-1).astype(np.float64).sum())
    loss = np.float32(total / BATCH)
    return np.asarray(loss, dtype=np.float32)

